# revision 1
# baseline (speedup 1.0000x reference)
# CRF loss (negative log-likelihood) kernel for Trainium2 (Bass/Tile).
#
# Algorithm: scaled linear-domain forward recursion.
#   fs_t = logsumexp_i(fs_{t-1}[i] + T[i,j]) + e_t[j]   (per batch row)
# is computed as v_t = (expT^T @ v_{t-1}) * exp(e_t - kappa), where
# v_t = exp(fs_t - kappa*(t+1)).  With kappa ~ log(L)+0.5 the state stays
# within e^{+-12} over all 512 steps for N(0,1) emissions, so no
# renormalisation is needed (validated against the f64 reference:
# rel err ~1e-7).  The per-batch sequence end (variable lengths) is
# handled by a one-hot selection mask over time, so the whole program is
# SPMD across the 8 cores (pure batch data-parallelism, 32 rows/core).
#
# Gold path score is computed on device as
#   sum_masked emit[s,b,lab] = sum((lab==l_partition) * raw_emit)  (one-hot dot)
#   sum T[prev,nxt] + sum T[end,PAD] = sum(T * C)  with C a host-built
#   count matrix (pure integer bookkeeping from labels/masks).
# The START->lab[0] transition T[1, lab0] is folded into emit[0] on the
# host and therefore excluded from C.

import os
import numpy as np

S, B, L = 512, 256, 128
NCORES = 8
BL = B // NCORES          # 32 batch rows per core
CH = 32                   # time steps per processing chunk
NCH = S // CH             # 16 chunks
PAD, START = 0, 1
KAPPA = float(np.log(L) + 0.5)

_PROGRAM = None
TRACE = False          # set by test harness to capture an NTFF profile
LAST_RESULTS = None    # BassKernelResults of the most recent kernel() call


def _build_program():
    import concourse.bass as bass
    import concourse.tile as tile
    from concourse import bacc, mybir

    f32 = mybir.dt.float32
    nc = bacc.Bacc(
        "TRN2",
        target_bir_lowering=False,
        debug=False,
        enable_asserts=False,
        num_devices=NCORES,
    )

    emitT = nc.dram_tensor("emitT", [L, S, BL], f32, kind="ExternalInput").ap()
    Tm = nc.dram_tensor("T", [L, L], f32, kind="ExternalInput").ap()
    labm = nc.dram_tensor("labm", [S, BL], f32, kind="ExternalInput").ap()
    selm = nc.dram_tensor("selm", [S, BL], f32, kind="ExternalInput").ap()
    cmat = nc.dram_tensor("cmat", [L, L], f32, kind="ExternalInput").ap()
    ktb = nc.dram_tensor("ktb", [1, BL], f32, kind="ExternalInput").ap()
    loss_out = nc.dram_tensor("loss", [1, 1], f32, kind="ExternalOutput").ap()

    EXP = mybir.ActivationFunctionType.Exp
    LN = mybir.ActivationFunctionType.Ln
    ADD = mybir.AluOpType.add
    MULT = mybir.AluOpType.mult
    ISEQ = mybir.AluOpType.is_equal
    AXX = mybir.AxisListType.X

    def bcast128(src_ap):
        # partition-broadcast a DRAM access pattern across 128 partitions
        return bass.AP(
            tensor=src_ap.tensor, offset=src_ap.offset, ap=[[0, 128]] + src_ap.ap
        )

    with tile.TileContext(nc) as tc:
        with (
            tc.tile_pool(name="singles", bufs=1) as singles,
            tc.tile_pool(name="raws", bufs=3) as raws,
            tc.tile_pool(name="labs", bufs=3) as labs,
            tc.tile_pool(name="sels", bufs=3) as sels,
            tc.tile_pool(name="junk", bufs=2) as junk,
            tc.tile_pool(name="selp", bufs=2) as selp,
            tc.tile_pool(name="psums", bufs=6, space="PSUM") as psums,
            tc.tile_pool(name="psum1", bufs=1, space="PSUM") as psum1,
        ):
            # ---------------- persistent state ----------------
            v_all = singles.tile([128, S * BL], f32)     # forward state history
            E_all = singles.tile([128, S * BL], f32)     # exp(e_t - kappa)
            gold_cols = singles.tile([128, NCH + 1], f32)
            Racc = singles.tile([128, CH * BL], f32)     # selected-state accum
            nc.gpsimd.memset(Racc, 0.0)

            # ---------------- constants ----------------
            T_sb = singles.tile([128, L], f32)
            nc.sync.dma_start(out=T_sb, in_=Tm[:, :])
            expT = singles.tile([128, L], f32)
            nc.scalar.activation(out=expT, in_=T_sb, func=EXP)
            cm_sb = singles.tile([128, L], f32)
            nc.sync.dma_start(out=cm_sb, in_=cmat[:, :])
            ktb_sb = singles.tile([1, BL], f32)
            nc.sync.dma_start(out=ktb_sb, in_=ktb[:, :])
            ones_col = singles.tile([128, 1], f32)
            nc.vector.memset(ones_col, 1.0)
            iota_i = singles.tile([128, 1], mybir.dt.int32)
            nc.gpsimd.iota(iota_i, pattern=[[0, 1]], channel_multiplier=1)
            iota_col = singles.tile([128, 1], f32)
            nc.gpsimd.tensor_copy(out=iota_col, in_=iota_i)
            negk = singles.tile([128, 1], f32)
            nc.vector.memset(negk, -KAPPA)

            # gold: sum(T * C) -> gold_cols[:, NCH]
            tc_junk = junk.tile([128, L], f32, tag="junk")
            nc.vector.scalar_tensor_tensor(
                out=tc_junk, in0=T_sb, scalar=1.0, in1=cm_sb,
                op0=MULT, op1=MULT,
                accum_out=gold_cols[:, NCH:NCH + 1],
            )

            # ---------------- main loop over time chunks ----------------
            for k in range(NCH):
                t0 = k * CH
                raw = raws.tile([128, CH * BL], f32, tag="raw")
                nc.sync.dma_start(out=raw, in_=emitT[:, t0:t0 + CH, :])

                # E = exp(raw - kappa); chunk 0's first step is the initial
                # state v_0 = exp(e_0 + T[START,:] - kappa) (T row folded on host)
                if k == 0:
                    nc.scalar.activation(
                        out=v_all[:, 0:BL], in_=raw[:, 0:BL], func=EXP, bias=negk
                    )
                    nc.scalar.activation(
                        out=E_all[:, BL:CH * BL], in_=raw[:, BL:CH * BL],
                        func=EXP, bias=negk,
                    )
                else:
                    nc.scalar.activation(
                        out=E_all[:, t0 * BL:(t0 + CH) * BL], in_=raw,
                        func=EXP, bias=negk,
                    )

                # gold emissions one-hot dot (GpSimd engine, off chain):
                # sum over chunk of raw[l,(t,b)] * (labm[(t,b)] == l)
                lab = labs.tile([128, CH * BL], f32, tag="lab")
                nc.gpsimd.dma_start(out=lab, in_=bcast128(labm[t0:t0 + CH, :]))
                oh_junk = junk.tile([128, CH * BL], f32, tag="junk")
                nc.vector.scalar_tensor_tensor(
                    out=oh_junk, in0=lab, scalar=iota_col, in1=raw,
                    op0=ISEQ, op1=MULT,
                    accum_out=gold_cols[:, k:k + 1],
                )

                # ---- the sequential chain for this chunk ----
                for t in range(max(t0, 1), t0 + CH):
                    ps = psums.tile([128, BL], f32, tag="ps")
                    nc.tensor.matmul(
                        ps, lhsT=expT, rhs=v_all[:, (t - 1) * BL:t * BL],
                        start=True, stop=True,
                    )
                    nc.vector.tensor_mul(
                        v_all[:, t * BL:(t + 1) * BL], ps,
                        E_all[:, t * BL:(t + 1) * BL],
                    )

                # selection (GpSimd, off chain): Racc += v * selmask
                selb = sels.tile([128, CH * BL], f32, tag="selb")
                nc.gpsimd.dma_start(out=selb, in_=bcast128(selm[t0:t0 + CH, :]))
                sp = selp.tile([128, CH * BL], f32, tag="sp")
                nc.gpsimd.tensor_mul(
                    sp, v_all[:, t0 * BL:(t0 + CH) * BL], selb
                )
                nc.gpsimd.tensor_add(Racc, Racc, sp)

            # ---------------- epilogue (emitted last = lowest priority) ----
            # reduce accumulated selection over time -> selected state V[l, b]
            Rsel = singles.tile([128, BL], f32)
            nc.vector.tensor_reduce(
                out=Rsel,
                in_=Racc.rearrange("p (t b) -> p b t", b=BL),
                axis=AXX, op=ADD,
            )
            # weight by exp(T[:, PAD]) and reduce over partitions via matmul
            W = singles.tile([128, BL], f32)
            nc.vector.tensor_scalar_mul(W, Rsel, expT[:, 0:1])
            r_ps = psum1.tile([1, BL], f32, tag="rps")
            nc.tensor.matmul(r_ps, lhsT=ones_col, rhs=W, start=True, stop=True)
            enc_row = singles.tile([1, BL], f32)
            nc.scalar.activation(out=enc_row, in_=r_ps, func=LN)
            enc_f = singles.tile([1, BL], f32)
            nc.vector.tensor_add(enc_f, enc_row, ktb_sb)
            enc_1 = singles.tile([1, 1], f32)
            nc.vector.tensor_reduce(out=enc_1, in_=enc_f, axis=AXX, op=ADD)

            gold_col = singles.tile([128, 1], f32)
            nc.vector.tensor_reduce(out=gold_col, in_=gold_cols, axis=AXX, op=ADD)
            g_ps = psum1.tile([1, 1], f32, tag="gps")
            nc.tensor.matmul(g_ps, lhsT=ones_col, rhs=gold_col, start=True, stop=True)

            loss_sb = singles.tile([1, 1], f32)
            nc.vector.tensor_sub(loss_sb, enc_1, g_ps)
            nc.sync.dma_start(out=loss_out[:, :], in_=loss_sb)

    nc.compile()
    return nc


def _get_program():
    global _PROGRAM
    if _PROGRAM is None:
        _PROGRAM = _build_program()
    return _PROGRAM


def _host_inputs(emit, labels, masks, T):
    """Per-core input maps (host-side sharding + index bookkeeping)."""
    lengths = masks.astype(np.int64).sum(axis=1)  # (B,)
    in_maps = []
    for c in range(NCORES):
        bsl = slice(c * BL, (c + 1) * BL)
        emitT = np.ascontiguousarray(emit[:, bsl, :].transpose(2, 0, 1))  # (L,S,BL)
        emitT[:, 0, :] += T[START, :][:, None]
        lab = labels[bsl]            # (BL, S) int32
        msk = masks[bsl]             # (BL, S) bool
        lens = lengths[bsl]          # (BL,)

        labm = lab.T.astype(np.float32).copy()     # (S, BL)
        labm[~msk.T] = 1000.0
        selmask = np.zeros((S, BL), np.float32)
        selmask[lens - 1, np.arange(BL)] = 1.0

        Cm = np.zeros((L, L), np.float32)
        prev = lab[:, :-1]
        nxt = lab[:, 1:]
        m2 = msk[:, 1:]
        np.add.at(Cm, (prev[m2], nxt[m2]), 1.0)
        ends = lab[np.arange(BL), lens - 1]
        np.add.at(Cm, (ends, np.full(BL, PAD)), 1.0)

        ktb_row = (KAPPA * lens.astype(np.float64)).astype(np.float32)[None, :]
        in_maps.append({
            "emitT": emitT,
            "T": np.ascontiguousarray(T, dtype=np.float32),
            "labm": labm,
            "selm": selmask,
            "cmat": Cm,
            "ktb": ktb_row,
        })
    return in_maps


def kernel(emit_scores, labels, masks, T):
    from concourse.bass_utils import run_bass_kernel_spmd

    emit = np.asarray(emit_scores, dtype=np.float32)
    labels = np.asarray(labels)
    masks = np.asarray(masks)
    T = np.asarray(T, dtype=np.float32)

    nc = _get_program()
    in_maps = _host_inputs(emit, labels, masks, T)
    res = run_bass_kernel_spmd(
        nc, in_maps, core_ids=list(range(NCORES)), trace=TRACE
    )
    global LAST_RESULTS
    LAST_RESULTS = res
    total = np.float64(0.0)
    for r in res.results:
        total += np.float64(r["loss"][0, 0])
    return np.asarray(total, dtype=np.float32)

